# revision 1
# baseline (speedup 1.0000x reference)
"""Piecewise-linear (hat-function) basis kernel for TRN2.

out[n, k] = relu(1 - |scaled_n - k|),  scaled = (clip(x,-1,1) + 1) * 63.5
which equals the reference's one_hot/floor/lerp formulation exactly in
exact arithmetic (and to ~3e-8 absmax in fp32).

Sharding: flat input axis split evenly across 8 cores (data parallel).
Per core: 131072 elements -> SBUF layout [128 partitions x 1024 cols],
partition p owns output rows p*1024 .. p*1024+1023.

Compute: a runtime-registered custom DVE op ANT_HAT_PWL2 computing
relu(1 - |in0 - in1|) in one fused pass over a paged [P, 64, 128] view:
in0 = knot row broadcast across pages (page-stride 0), in1 = scaled
inputs broadcast across knots (elem-stride 0). One instruction per
64-column group (16 total), each followed by a 4MB DMA of that group's
[128, 8192] tile to DRAM; 4-buffer ring overlaps compute with DMA.
Bacc.finalize() legalizes multi-wait syncs (HW allows 1 wait per
instruction; EventSemaphore 2).
"""

import numpy as np

import concourse.bacc as bacc
import concourse.bass as bass
import concourse.mybir as mybir
from concourse import dve_ops
from concourse.bass_utils import run_bass_kernel_spmd
from concourse.dve_spec import One, Spec, Src0, Src1, _has_src1, lower, maxx, relu
from concourse.dve_uop import DveOpSpec
from concourse.tile import TileContext

N = 1048576
K = 128
NCORES = 8
N_CORE = N // NCORES  # 131072
P = 128
C = N_CORE // P  # 1024 element-columns per partition
GROUP = 64  # element-columns per compute/DMA batch
NGROUPS = C // GROUP  # 16
NBUF = 4
RSTEP = 63.5  # fl32(1/fl32(2/127)) == 63.5 exactly

F32 = mybir.dt.float32
Alu = mybir.AluOpType

_HAT2_SPEC = Spec(
    body=relu(One - maxx(Src0 - Src1, Src1 - Src0)),
    reference=lambda in0, in1, s0, s1, imm2: np.maximum(
        1.0 - np.abs(in0 - in1), 0.0
    ).astype(np.float32),
)


def _register_hat2() -> dve_ops.DveOp:
    name = "ANT_HAT_PWL2"
    if name in dve_ops._SUB_OPCODE_FOR_NAME:
        return next(op for op in dve_ops.OPS if op.name == name)
    row = max(dve_ops._SUB_OPCODE_FOR_NAME.values()) + 1
    assert row < 0x20, row
    dve_ops._SUB_OPCODE_FOR_NAME[name] = row
    shas = {
        ver: DveOpSpec(
            name=name,
            opcode=row,
            uops=lower(_HAT2_SPEC, ver=ver),
            rd1_en=_has_src1(_HAT2_SPEC),
        ).sha(ver)
        for ver in ("v3", "v4")
    }
    op = dve_ops.DveOp(name, _HAT2_SPEC, subdim=False, uops_sha=shas)
    dve_ops.OPS.append(op)
    dve_ops.CUSTOM_DVE_SPECS[name] = _HAT2_SPEC
    return op


HAT2 = _register_hat2()


def _build() -> bass.Bass:
    # Bacc (not raw Bass): its finalize() runs generate_event_semaphores,
    # which splits multi-wait syncs to satisfy the 1-wait HW constraint.
    nc = bacc.Bacc("TRN2", target_bir_lowering=False, debug=False)
    xk = nc.dram_tensor("xk", [P, C + K], F32, kind="ExternalInput")
    out = nc.dram_tensor("out", [N_CORE, K], F32, kind="ExternalOutput")

    out2 = out.rearrange("(p c) k -> p (c k)", p=P)  # [128, 131072]

    with TileContext(nc) as tc:
        with tc.tile_pool(name="persist", bufs=1) as ppool:
            xs = ppool.tile([P, C + K], F32, name="xs")
            s = ppool.tile([P, C], F32, name="s")
            bufs = [
                ppool.tile([P, GROUP * K], F32, name=f"b{i}") for i in range(NBUF)
            ]

            nc.gpsimd.dma_start(out=xs, in_=xk[:])
            kn = xs[:, C : C + K]  # knot row 0..127, replicated host-side
            in0 = kn.unsqueeze(1).broadcast_to([P, GROUP, K])

            # clamp to [-1, 1], then scaled = (c + 1) * 63.5
            nc.vector.tensor_scalar(s, xs[:, 0:C], -1.0, 1.0, Alu.max, Alu.min)
            nc.vector.tensor_scalar(s, s, 1.0, RSTEP, Alu.add, Alu.mult)

            for g in range(NGROUPS):
                B = bufs[g % NBUF]
                in1 = (
                    s[:, g * GROUP : (g + 1) * GROUP]
                    .unsqueeze(2)
                    .broadcast_to([P, GROUP, K])
                )
                o3 = B[:].rearrange("p (g k) -> p g k", g=GROUP)
                nc.vector._custom_dve(HAT2, out=o3, in0=in0, in1=in1)
                nc.sync.dma_start(
                    out=out2[:, g * GROUP * K : (g + 1) * GROUP * K],
                    in_=B[:],
                )
    nc.finalize()
    return nc


def _in_maps(x: np.ndarray) -> list[dict]:
    knot_row = np.broadcast_to(np.arange(K, dtype=np.float32)[None, :], (P, K))
    shards = x.reshape(NCORES, P, C)
    return [
        {"xk": np.ascontiguousarray(np.concatenate([shards[i], knot_row], axis=1))}
        for i in range(NCORES)
    ]


def kernel(inputs: np.ndarray, num_knots) -> np.ndarray:
    assert int(num_knots) == K, f"kernel hardcoded for num_knots={K}"
    x = np.ascontiguousarray(np.asarray(inputs, dtype=np.float32))
    assert x.shape == (N,), x.shape

    nc = _build()
    res = run_bass_kernel_spmd(nc, _in_maps(x), core_ids=list(range(NCORES)))
    return np.concatenate([r["out"] for r in res.results], axis=0)

